# revision 37
# baseline (speedup 1.0000x reference)
import sys

for p in ("/opt/trn_rl_repo",):
    if p not in sys.path:
        sys.path.insert(0, p)

import numpy as np
import ml_dtypes

import concourse.bass as bass
from concourse import bacc
import concourse.mybir as mybir
import concourse.tile as tile
from concourse.bass import ds, ts
from concourse.bass_utils import run_bass_kernel_spmd

BF16 = ml_dtypes.bfloat16

B, N, DIM, NH = 256, 196, 256, 8
HD = DIM // NH  # 32
G = 14
NCORES = 8
BLOC = B // NCORES  # 32
NC2 = 98  # N / 2

# Heads whose exp+bias runs as the bf16-bit-space affine trick on the DVE
# instead of ACT exp + separate multiply. Tunable for engine balance.
TRICK_HEADS = ()
EXP_A = 128.0 / float(np.log(2.0))  # bf16-bit-space exp scale
EXP_B0 = 16250.0


def _relative_position_index(g: int) -> np.ndarray:
    coords = np.stack(np.meshgrid(np.arange(g), np.arange(g), indexing="ij"))
    cf = coords.reshape(2, -1)
    rel = cf[:, :, None] - cf[:, None, :]
    rel = rel.transpose(1, 2, 0).astype(np.int64)
    rel[..., 0] += g - 1
    rel[..., 1] += g - 1
    rel[..., 0] *= 2 * g - 1
    return rel.sum(-1)


def _bias_coords(g: int) -> np.ndarray:
    p = np.arange(1 - g, g)
    biases = np.stack(np.meshgrid(p, p, indexing="ij"))
    return biases.reshape(2, -1).T.astype(np.float32)


_CACHED = {}


def _build_bass():
    if "nc" in _CACHED:
        return _CACHED["nc"]
    f32 = mybir.dt.float32
    bf16 = mybir.dt.bfloat16
    i16 = mybir.dt.int16

    nc = bacc.Bacc("TRN2", target_bir_lowering=False)
    qt_d = nc.dram_tensor("qt", [BLOC, 32, 8, 196], bf16, kind="ExternalInput")
    kt_d = nc.dram_tensor("kt", [BLOC, 32, 8, 196], bf16, kind="ExternalInput")
    vx_d = nc.dram_tensor("vx", [BLOC, NC2, 2, 8, 33], bf16, kind="ExternalInput")
    erpb_d = nc.dram_tensor("erpb", [NC2, 2, 8, 196], bf16, kind="ExternalInput")
    bmat_d = nc.dram_tensor("bmat", [NC2, 2, 8, 196], f32, kind="ExternalInput")
    w_d = nc.dram_tensor("w", [128, 2, 256], bf16, kind="ExternalInput")
    pb_d = nc.dram_tensor("pb", [NC2, 256], f32, kind="ExternalInput")
    ident_d = nc.dram_tensor("ident", [NC2, NC2], bf16, kind="ExternalInput")
    out_d = nc.dram_tensor("out", [BLOC, 196, 256], f32, kind="ExternalOutput")

    from contextlib import ExitStack

    with tile.TileContext(nc) as tc, ExitStack() as es:
        const = es.enter_context(tc.tile_pool(name="const", bufs=1))
        io = es.enter_context(tc.tile_pool(name="io", bufs=3))
        work = es.enter_context(tc.tile_pool(name="work", bufs=3))
        psum_s = es.enter_context(tc.tile_pool(name="psum_s", bufs=3, space="PSUM"))
        psum_xp = es.enter_context(tc.tile_pool(name="psum_x", bufs=3, space="PSUM"))
        psum_tp = es.enter_context(tc.tile_pool(name="psum_t", bufs=1, space="PSUM"))
        psum_op = es.enter_context(tc.tile_pool(name="psum_o", bufs=1, space="PSUM"))

        # qt/kt live in 98-partition tiles with rows 32..97 zeroed: a 98-row
        # stationary loads ~2x faster than a 32-row one, and zero rows don't
        # change the contraction. Rows are zeroed once per ring slot; the
        # per-batch DMA only ever writes rows 0..31.
        def dma_in(b):
            # split the first batches' loads across rings to cut the
            # startup-latency; steady-state single DMAs are cheaper overall
            nchunk = 4 if b < 2 else 1
            qt_sb = io.tile([NC2, 8, 196], bf16, tag="qt")
            kt_sb = io.tile([NC2, 8, 196], bf16, tag="kt")
            for c in range(0, 8, 8 // nchunk):
                ce = c + 8 // nchunk
                nc.sync.dma_start(qt_sb[0:32, c:ce], qt_d[b, :, c:ce])
                nc.sync.dma_start(kt_sb[0:32, c:ce], kt_d[b, :, c:ce])
            vx_sb = io.tile([NC2, 2, 8, 33], bf16, tag="vx")
            if b < 2:
                for j in range(2):
                    nc.sync.dma_start(vx_sb[:, j], vx_d[b, :, j])
            else:
                nc.sync.dma_start(vx_sb[:], vx_d[b])
            return qt_sb, kt_sb, vx_sb

        # zero each qt/kt ring slot once (memset has a fixed ~1.35us cost
        # regardless of size, so one full-tile op per slot, DVE and Pool in
        # parallel, slot 0 first); the per-batch DMA overwrites rows 0..31
        for s in range(3):
            t = io.tile([NC2, 8, 196], bf16, tag="qt")
            nc.vector.memset(t[:], 0.0)
            t = io.tile([NC2, 8, 196], bf16, tag="kt")
            nc.gpsimd.memset(t[:], 0.0)

        # prefetch batch 0/1 inputs BEFORE the const tables so their ring
        # slots aren't queued behind the 11 const DMAs
        in01 = [dma_in(0), dma_in(1)]

        # split big/latency-critical DMAs across rings (one ring ~17 GB/s)
        erpb_sb = const.tile([NC2, 2, 8, 196], bf16)
        for h in range(8):
            nc.sync.dma_start(erpb_sb[:, :, h], erpb_d[:, :, h])
        if TRICK_HEADS:
            bmat_sb = const.tile([NC2, 2, 8, 196], f32)
            nc.sync.dma_start(bmat_sb[:], bmat_d[:])
        w_sb = const.tile([128, 2, 256], bf16)
        nc.sync.dma_start(w_sb[:], w_d[:])
        pb_sb = const.tile([NC2, 256], f32)
        nc.sync.dma_start(pb_sb[:], pb_d[:])
        ident_sb = const.tile([NC2, NC2], bf16)
        nc.sync.dma_start(ident_sb[:], ident_d[:])

        tail_state = {}

        def tail():
            # transpose x [n, c] -> xt [c, n] on the PE (identity matmul),
            # then proj + bias + store. Runs one batch late, inside the next
            # batch's QK wave, so its weight-loads hide under long streams.
            b0 = tail_state.pop("b")
            xv = tail_state.pop("x").rearrange("p i h d -> p i (h d)")
            xt_sbs = []
            for half in range(2):
                xtp = psum_tp.tile([128, 2, NC2], bf16, tag="xtp")
                for i in range(2):
                    nc.tensor.transpose(
                        xtp[:, i],
                        xv[:, i, ts(half, 128)],
                        ident_sb[:],
                    )
                xt_sb = work.tile([128, 2, NC2], bf16, tag=f"xt{half}")
                if half == 0:
                    nc.scalar.copy(xt_sb[:], xtp[:])
                else:
                    nc.vector.tensor_copy(xt_sb[:], xtp[:])
                xt_sbs.append(xt_sb)

            for i in range(2):
                po = psum_op.tile([NC2, 256], f32, tag="po")
                for half in range(2):
                    nc.tensor.matmul(
                        po[:],
                        lhsT=xt_sbs[half][:, i],
                        rhs=w_sb[:, half],
                        start=(half == 0),
                        stop=(half == 1),
                    )
                o_sb = work.tile([NC2, 256], f32, tag="o", name=f"o{i}")
                nc.vector.tensor_add(out=o_sb[:], in0=po[:], in1=pb_sb[:])
                if b0 >= BLOC - 2:
                    # split the final stores across rings to cut tail drain
                    nc.sync.dma_start(
                        out_d[b0, ds(i * NC2, NC2), 0:128], o_sb[:, 0:128]
                    )
                    nc.sync.dma_start(
                        out_d[b0, ds(i * NC2, NC2), 128:256], o_sb[:, 128:256]
                    )
                else:
                    nc.sync.dma_start(out_d[b0, ds(i * NC2, NC2)], o_sb[:])

        for b in range(BLOC):
            qt_sb, kt_sb, vx_sb = in01[b] if b < 2 else dma_in(b)

            x_sb = work.tile([NC2, 2, 8, 32], bf16, tag="x")
            psum_x = [
                psum_xp.tile([NC2, 8, 33], f32, tag="px", name=f"px{i}")
                for i in range(2)
            ]

            # wave schedule: 4 QK heads issued back-to-back so every
            # weight-load hides under a long stream, and PV's stationary
            # (pst) is dep-ready well before its matmul issues
            ps_tiles = {}

            def qk(h):
                ps = psum_s.tile([NC2, 2, 196], f32, tag="ps")
                for j in range(2):
                    nc.tensor.matmul(
                        ps[:, j],
                        lhsT=kt_sb[:, h, ts(j, NC2)],
                        rhs=qt_sb[:, h, :],
                        start=True,
                        stop=True,
                    )
                ps_tiles[h] = ps

            for h in range(4):
                qk(h)
            if tail_state:
                tail()
            for h in range(8):
                if h < 4:
                    qk(h + 4)
                ps = ps_tiles.pop(h)
                pst = work.tile([NC2, 2, 196], bf16, tag="pst")
                if h in TRICK_HEADS:
                    # trick reads PSUM -> DVE only (GpSimd cannot access PSUM)
                    eng = nc.vector
                    eng.scalar_tensor_tensor(
                        out=pst[:].bitcast(i16),
                        in0=ps[:],
                        scalar=EXP_A,
                        in1=bmat_sb[:, :, h],
                        op0=mybir.AluOpType.mult,
                        op1=mybir.AluOpType.add,
                    )
                else:
                    est = work.tile([NC2, 2, 196], bf16, tag="est")
                    nc.scalar.activation(
                        est[:], ps[:], mybir.ActivationFunctionType.Exp
                    )
                    # SBUF-only multiply; Pool runs ~2.7x slower than DVE, so
                    # it only takes 3 of 6 (DVE also owns the PSUM-input ops)
                    eng = nc.gpsimd if h in (0, 1, 2) else nc.vector
                    eng.tensor_mul(out=pst[:], in0=est[:], in1=erpb_sb[:, :, h])
                for i in range(2):
                    for j in range(2):
                        nc.tensor.matmul(
                            psum_x[i][:, h],
                            lhsT=pst[:, j, ts(i, NC2)],
                            rhs=vx_sb[:, j, h],
                            start=(j == 0),
                            stop=(j == 1),
                        )

            for i in range(2):
                rc = work.tile([NC2, 8], f32, tag="rc")
                nc.vector.reciprocal(rc[:], psum_x[i][:, :, 32])
                nc.vector.tensor_mul(
                    out=x_sb[:, i],
                    in0=psum_x[i][:, :, 0:32],
                    in1=rc[:, :, None].to_broadcast([NC2, 8, 32]),
                )
            tail_state.update(b=b, x=x_sb)

        tail()

    nc.compile()
    _CACHED["nc"] = nc
    return nc


def _prep_host(q, k, v, dpb_w1, dpb_b1, dpb_w2, dpb_b2, proj_w, proj_b):
    scale = HD ** -0.5
    # qT/kT packed [B, 32, 8, 196]: [p, h, n] = q[b, n, h*32+p]
    qs = (q.astype(np.float32) * scale).transpose(0, 2, 1).reshape(B, 8, 32, 196)
    qt = np.ascontiguousarray(qs.transpose(0, 2, 1, 3)).astype(BF16)
    ks = k.astype(np.float32).transpose(0, 2, 1).reshape(B, 8, 32, 196)
    kt = np.ascontiguousarray(ks.transpose(0, 2, 1, 3)).astype(BF16)
    # v ext [B, 98, 2, 8, 33] with a trailing ones column for the row sums
    vr = v.reshape(B, 2, NC2, 8, 32).transpose(0, 2, 1, 3, 4)
    vx = np.concatenate([vr, np.ones(vr.shape[:-1] + (1,), np.float32)], axis=-1)
    vx = np.ascontiguousarray(vx).astype(BF16)
    # rpb via MLP on host
    biases = _bias_coords(G)
    pos = np.maximum(biases @ dpb_w1 + dpb_b1, 0.0) @ dpb_w2 + dpb_b2  # [729, 8]
    idx = _relative_position_index(G).reshape(-1)
    rpb = pos[idx].reshape(N, N, NH).transpose(2, 0, 1)  # [H, n, m]
    # erpb [98, 2, 8, 196]: [p, j, h, n] = exp(rpb[h, n, j*98+p])
    er = np.exp(rpb).transpose(2, 0, 1)  # [m, h, n]
    erpb = np.ascontiguousarray(
        er.reshape(2, NC2, 8, 196).transpose(1, 0, 2, 3)
    ).astype(BF16)
    # bmat: same layout, affine bias for the bf16-bit-space exp trick
    bm = (EXP_B0 + EXP_A * rpb).transpose(2, 0, 1)  # [m, h, n]
    bmat = np.ascontiguousarray(
        bm.reshape(2, NC2, 8, 196).transpose(1, 0, 2, 3)
    ).astype(np.float32)
    w = np.ascontiguousarray(
        proj_w.reshape(2, 128, 256).transpose(1, 0, 2)
    ).astype(BF16)
    pb = np.broadcast_to(proj_b.reshape(1, 256), (NC2, 256)).astype(np.float32)
    pb = np.ascontiguousarray(pb)
    ident = np.eye(NC2, dtype=np.float32).astype(BF16)
    return qt, kt, vx, erpb, bmat, w, pb, ident


def kernel(**inputs) -> np.ndarray:
    q = np.asarray(inputs["q"], np.float32)
    k = np.asarray(inputs["k"], np.float32)
    v = np.asarray(inputs["v"], np.float32)
    qt, kt, vx, erpb, bmat, w, pb, ident = _prep_host(
        q, k, v,
        np.asarray(inputs["dpb_w1"], np.float32),
        np.asarray(inputs["dpb_b1"], np.float32),
        np.asarray(inputs["dpb_w2"], np.float32),
        np.asarray(inputs["dpb_b2"], np.float32),
        np.asarray(inputs["proj_w"], np.float32),
        np.asarray(inputs["proj_b"], np.float32),
    )
    nc = _build_bass()
    in_maps = []
    for c in range(NCORES):
        sl = slice(c * BLOC, (c + 1) * BLOC)
        in_maps.append(
            {
                "qt": np.ascontiguousarray(qt[sl]),
                "kt": np.ascontiguousarray(kt[sl]),
                "vx": np.ascontiguousarray(vx[sl]),
                "erpb": erpb,
                "bmat": bmat,
                "w": w,
                "pb": pb,
                "ident": ident,
            }
        )
    res = run_bass_kernel_spmd(
        nc, in_maps, core_ids=list(range(NCORES)), trace=bool(_CACHED.get("trace"))
    )
    _CACHED["last_results"] = res
    out = np.concatenate([r["out"] for r in res.results], axis=0)
    return out.astype(np.float32)


if __name__ == "__main__":
    rng = np.random.default_rng(0)
    ins = {
        "q": rng.standard_normal((B, N, DIM), dtype=np.float32),
        "k": rng.standard_normal((B, N, DIM), dtype=np.float32),
        "v": rng.standard_normal((B, N, DIM), dtype=np.float32),
        "dpb_w1": rng.standard_normal((2, 64), dtype=np.float32) * 0.1,
        "dpb_b1": np.zeros(64, np.float32),
        "dpb_w2": rng.standard_normal((64, 8), dtype=np.float32) * 0.1,
        "dpb_b2": np.zeros(8, np.float32),
        "proj_w": rng.standard_normal((256, 256), dtype=np.float32) * (256 ** -0.5),
        "proj_b": np.zeros(256, np.float32),
        "group_size": 14,
    }
    o = kernel(**ins)
    print(o.shape, o.dtype)
